# revision 3
# baseline (speedup 1.0000x reference)
"""Trainium2 Bass kernel for per-edge dot products (DGL u_dot_v).

score[e] = sum_d h[src[e], d] * h[dst[e], d]   for 640K edges, 10K nodes, D=128.

Strategy (8 NeuronCores, data-parallel over edges, 80K edges/core):

Per-edge gathers on-device are descriptor/ucode-rate bound on this part
(SWDGE dma_gather and gpsimd ap_gather both cost 100s of ns/edge-endpoint
in instruction-issue terms), while the DMA engines stream sequential data
at full rate. So the host lays the gathered operands out as bf16
edge-major slabs in HBM and the device runs a pure streaming pipeline:

  - Host: sort each core's edges by src and pad equal-src runs to even
    length (v1's pairing); pair p shares one hu entry between its two
    edges -> hu slab is half size (25% total DMA saved). Slot layout is
    "halves": pair p = (lane p%128, group p//128) and (same lane,
    group p//128 + NPG), so every device access stays packed-contiguous.
  - DMA: stream hu2 [128, tile, 128] and hv [128, 2, tile, 128] tiles.
  - DVE: two muls (hu2 broadcast across the two halves) in bf16 2x mode.
  - DVE: feature reduction as a binary tree of tensor_tensor adds over
    contiguous half-splits (2x mode per level) — tensor_reduce has no
    fast mode (1 elem/cycle) and would dominate.
  - One f32 scores [128, 672] tile, single DMA out; host inverts the
    permutation.

Measured ~110us/core steady-state on hardware (TimelineSim models 112us);
the DMA stream (33.6MB/core/pass at ~330GB/s) and DVE (~86K cycles) are
both near-saturated.
"""

import sys

import numpy as np

for _p in ("/opt/trn_rl_repo", "/opt/pypackages"):
    if _p not in sys.path:
        sys.path.append(_p)

import ml_dtypes  # noqa: E402

import concourse.mybir as mybir  # noqa: E402
import concourse.tile as tile  # noqa: E402
from concourse import bacc  # noqa: E402
from concourse.bass_utils import run_bass_kernel_spmd  # noqa: E402

N_NODES = 10000
D_FEAT = 128
N_EDGES = 640000
N_CORES = 8
E_PER = N_EDGES // N_CORES  # 80000
E2 = 86016  # padded slots per core (multiple of 256, fits worst pad)
NG = E2 // 128  # 672 slot groups
NPG = NG // 2  # 336 pair groups

_BUILT = {}


def build(loops=1, tile_g=21, bufs=4, pool_tiles=12):
    """Paired streaming kernel; tile_g = pair-groups per tile (divides 336).

    The half-size hu2 slab (11MB) is DMA'd once and stays SBUF-resident
    (86KB/partition); only hv streams per pass. pool_tiles tiles (from
    the end) run their second mul on GPSIMD to offload the DVE, which is
    otherwise the bottleneck. loops > 1 wraps the pass in a hardware
    For_i loop (identical output every iteration) so steady-state device
    time can be measured by loop-count differencing inside one NEFF."""
    key = ("p", loops, tile_g, bufs, pool_tiles)
    if key in _BUILT:
        return _BUILT[key]

    f32 = mybir.dt.float32
    bf16 = mybir.dt.bfloat16

    assert NPG % tile_g == 0
    n_tiles = NPG // tile_g

    nc = bacc.Bacc("TRN2", target_bir_lowering=False, debug=False)

    hu_d = nc.dram_tensor("hus", [128, NPG, D_FEAT], bf16, kind="ExternalInput")
    hv_d = nc.dram_tensor("hvs", [128, 2, NPG, D_FEAT], bf16, kind="ExternalInput")
    out_d = nc.dram_tensor("scores", [128, NG], f32, kind="ExternalOutput")

    with tile.TileContext(nc) as tc:
        with (
            tc.tile_pool(name="resid", bufs=1) as rpool,
            tc.tile_pool(name="outp", bufs=1) as outp,
            tc.tile_pool(name="stream", bufs=bufs) as gpool,
            tc.tile_pool(name="scratch", bufs=2) as spool,
        ):
            hu2 = rpool.tile([128, NPG, D_FEAT], bf16)
            nc.sync.dma_start(hu2[:], hu_d[:])
            scores = outp.tile([128, NG], f32)
            scores_v = scores[:].rearrange("p (r g) -> p r g", r=2)

            def body():
                for t in range(n_tiles):
                    g0 = t * tile_g
                    hv = gpool.tile([128, 2, tile_g, D_FEAT], bf16, tag="hv")
                    nc.sync.dma_start(hv[:], hv_d[:, :, g0 : g0 + tile_g, :])
                    prod = spool.tile([128, 2, tile_g, D_FEAT], bf16, tag="prod")
                    hu_sl = hu2[:, g0 : g0 + tile_g, :]
                    eng1 = nc.gpsimd if t >= n_tiles - pool_tiles else nc.vector
                    nc.vector.tensor_mul(prod[:, 0], hu_sl, hv[:, 0])
                    eng1.tensor_mul(prod[:, 1], hu_sl, hv[:, 1])
                    cur = prod
                    w = D_FEAT
                    while w > 2:
                        nxt = spool.tile(
                            [128, 2, tile_g, w // 2], bf16, tag=f"t{w}"
                        )
                        cv = cur[:].rearrange("p r g (h f) -> p r g h f", h=2)
                        nc.vector.tensor_add(
                            nxt[:], cv[:, :, :, 0, :], cv[:, :, :, 1, :]
                        )
                        cur = nxt
                        w //= 2
                    cv = cur[:].rearrange("p r g (h f) -> p r g h f", h=2)
                    nc.vector.tensor_add(
                        scores_v[:, :, g0 : g0 + tile_g],
                        cv[:, :, :, 0, 0],
                        cv[:, :, :, 1, 0],
                    )

            if loops == 1:
                body()
            else:
                with tc.For_i(0, loops, 1):
                    body()
            nc.sync.dma_start(out_d[:], scores[:])

    nc.compile()
    _BUILT[key] = nc
    return nc


def build_flat(loops=1, tile_g=125, bufs=2):
    """Unpaired fallback (no sorting): edge e at [e%128, e//128]."""
    key = ("f", loops, tile_g, bufs)
    if key in _BUILT:
        return _BUILT[key]

    f32 = mybir.dt.float32
    bf16 = mybir.dt.bfloat16

    n_groups = E_PER // 128  # 625
    assert n_groups % tile_g == 0
    n_tiles = n_groups // tile_g

    nc = bacc.Bacc("TRN2", target_bir_lowering=False, debug=False)

    hu_d = nc.dram_tensor("hus", [128, n_groups, D_FEAT], bf16, kind="ExternalInput")
    hv_d = nc.dram_tensor("hvs", [128, n_groups, D_FEAT], bf16, kind="ExternalInput")
    out_d = nc.dram_tensor("scores", [128, n_groups], f32, kind="ExternalOutput")

    with tile.TileContext(nc) as tc:
        with (
            tc.tile_pool(name="outp", bufs=1) as outp,
            tc.tile_pool(name="stream", bufs=bufs) as gpool,
            tc.tile_pool(name="prod", bufs=2) as ppool,
        ):
            scores = outp.tile([128, n_groups], f32)

            def body():
                for t in range(n_tiles):
                    g0 = t * tile_g
                    hu = gpool.tile([128, tile_g, D_FEAT], bf16, tag="hu")
                    hv = gpool.tile([128, tile_g, D_FEAT], bf16, tag="hv")
                    nc.sync.dma_start(hu[:], hu_d[:, g0 : g0 + tile_g, :])
                    nc.sync.dma_start(hv[:], hv_d[:, g0 : g0 + tile_g, :])
                    prod = ppool.tile([128, tile_g, D_FEAT], bf16)
                    nc.vector.tensor_mul(prod[:], hu[:], hv[:])
                    nc.vector.tensor_reduce(
                        scores[:, g0 : g0 + tile_g],
                        prod[:],
                        axis=mybir.AxisListType.X,
                        op=mybir.AluOpType.add,
                    )

            if loops == 1:
                body()
            else:
                with tc.For_i(0, loops, 1):
                    body()
            nc.sync.dma_start(out_d[:], scores[:])

    nc.compile()
    _BUILT[key] = nc
    return nc


def prep_paired(s, d, e2=E2):
    """Sort a core's edges by src, pad equal-src runs to even length.

    Returns (pair_src [e2/2], slot_dst [e2], ed_map [e2]) in
    pair-adjacent order (slots 2i, 2i+1 = pair i), or None on overflow.
    ed_map[j] = original edge index or -1 for padding."""
    n = len(s)
    order = np.argsort(s, kind="stable")
    ss, dd = s[order], d[order]
    change = np.flatnonzero(np.diff(ss)) + 1
    starts = np.concatenate(([0], change))
    ends = np.concatenate((change, [n]))
    lens = ends - starts
    odd = (lens % 2).astype(bool)
    if n + int(odd.sum()) > e2:
        return None
    pads_before = np.concatenate(([0], np.cumsum(odd)[:-1]))
    new_pos = np.arange(n) + np.repeat(pads_before, lens)
    psrc = np.zeros(e2, np.int64)
    pdst = np.zeros(e2, np.int64)
    pedge = np.full(e2, -1, np.int64)
    psrc[new_pos] = ss
    pdst[new_pos] = dd
    pedge[new_pos] = order
    pad_slots = (ends + pads_before)[odd]
    psrc[pad_slots] = ss[ends[odd] - 1]
    return psrc[0::2], pdst, pedge


def make_slabs(h_bf, pair_src, slot_dst):
    """hu2 [128, NPG, 128] (pair p at [p%128, p//128]); hv
    [128, 2, NPG, 128] (r=0: slots 2i, r=1: slots 2i+1)."""
    hu2 = np.ascontiguousarray(
        h_bf[pair_src].reshape(NPG, 128, D_FEAT).transpose(1, 0, 2)
    )
    hv = np.empty((128, 2, NPG, D_FEAT), h_bf.dtype)
    dst_pairs = slot_dst.reshape(NPG * 128, 2)
    for r in range(2):
        hv[:, r] = (
            h_bf[dst_pairs[:, r]].reshape(NPG, 128, D_FEAT).transpose(1, 0, 2)
        )
    return hu2, hv


def unscatter(scores, ed_map):
    """Device scores [128, NG] f32 -> per-original-edge [E_PER] f32."""
    p = np.arange(E2 // 2)
    lane = p % 128
    glo = p // 128
    flat = np.empty(E2, np.float32)
    flat[0::2] = scores[lane, glo]
    flat[1::2] = scores[lane, glo + NPG]
    valid = ed_map >= 0
    out_local = np.empty(E_PER, np.float32)
    out_local[ed_map[valid]] = flat[valid]
    return out_local


def make_slabs_flat(h_bf, src_k, dst_k):
    n_groups = E_PER // 128
    hus = np.ascontiguousarray(
        h_bf[src_k].reshape(n_groups, 128, D_FEAT).transpose(1, 0, 2)
    )
    hvs = np.ascontiguousarray(
        h_bf[dst_k].reshape(n_groups, 128, D_FEAT).transpose(1, 0, 2)
    )
    return hus, hvs


def kernel(h, src, dst):
    h_bf = np.asarray(h, dtype=np.float32).astype(ml_dtypes.bfloat16)
    src = np.asarray(src).astype(np.int64)
    dst = np.asarray(dst).astype(np.int64)

    preps = []
    for k in range(N_CORES):
        sl = slice(k * E_PER, (k + 1) * E_PER)
        preps.append(prep_paired(src[sl], dst[sl]))

    out = np.empty(N_EDGES, np.float32)
    if all(p is not None for p in preps):
        nc = build()
        in_maps = []
        for pair_src, slot_dst, _ in preps:
            hu2, hv = make_slabs(h_bf, pair_src, slot_dst)
            in_maps.append({"hus": hu2, "hvs": hv})
        res = run_bass_kernel_spmd(nc, in_maps, list(range(N_CORES)))
        for k in range(N_CORES):
            sc = res.results[k]["scores"].astype(np.float32)
            out[k * E_PER : (k + 1) * E_PER] = unscatter(sc, preps[k][2])
    else:
        nc = build_flat()
        in_maps = []
        for k in range(N_CORES):
            sl = slice(k * E_PER, (k + 1) * E_PER)
            hus, hvs = make_slabs_flat(h_bf, src[sl], dst[sl])
            in_maps.append({"hus": hus, "hvs": hvs})
        res = run_bass_kernel_spmd(nc, in_maps, list(range(N_CORES)))
        for k in range(N_CORES):
            sc = res.results[k]["scores"].astype(np.float32)
            out[k * E_PER : (k + 1) * E_PER] = sc.T.reshape(-1)
    return out.reshape(N_EDGES, 1)


# revision 5
# speedup vs baseline: 1.3834x; 1.3834x over previous
"""Trainium2 Bass kernel for per-edge dot products (DGL u_dot_v).

score[e] = sum_d h[src[e], d] * h[dst[e], d]   for 640K edges, 10K nodes, D=128.

Strategy (8 NeuronCores, data-parallel over edges, 80K edges/core):

Per-edge gathers on-device are descriptor/ucode-rate bound on this part
(SWDGE dma_gather and gpsimd ap_gather both cost 100s of ns/edge-endpoint
in instruction-issue terms), while the DMA engines stream sequential data
at full rate. So the host lays the gathered operands out as bf16
edge-major slabs in HBM and the device runs a pure streaming pipeline:

  - Host: sort each core's edges by src and pad equal-src runs to even
    length (v1's pairing); pair p shares one hu entry between its two
    edges -> hu slab is half size (25% total DMA saved). Slot layout is
    "halves": pair p = (lane p%128, group p//128) and (same lane,
    group p//128 + NPG), so every device access stays packed-contiguous.
  - DMA: stream hu2 [128, tile, 128] and hv [128, 2, tile, 128] tiles.
  - DVE: two muls (hu2 broadcast across the two halves) in bf16 2x mode.
  - DVE: feature reduction as a binary tree of tensor_tensor adds over
    contiguous half-splits (2x mode per level) — tensor_reduce has no
    fast mode (1 elem/cycle) and would dominate.
  - One f32 scores [128, 672] tile, single DMA out; host inverts the
    permutation.

Measured ~110us/core steady-state on hardware (TimelineSim models 112us);
the DMA stream (33.6MB/core/pass at ~330GB/s) and DVE (~86K cycles) are
both near-saturated.
"""

import sys

import numpy as np

for _p in ("/opt/trn_rl_repo", "/opt/pypackages"):
    if _p not in sys.path:
        sys.path.append(_p)

import ml_dtypes  # noqa: E402

import concourse.mybir as mybir  # noqa: E402
import concourse.tile as tile  # noqa: E402
from concourse import bacc  # noqa: E402
from concourse.bass_utils import run_bass_kernel_spmd  # noqa: E402

N_NODES = 10000
D_FEAT = 128
N_EDGES = 640000
N_CORES = 8
E_PER = N_EDGES // N_CORES  # 80000
E2 = 86016  # padded slots per core (multiple of 256, fits worst pad)
NG = E2 // 128  # 672 slot groups
NPG = NG // 2  # 336 pair groups

_BUILT = {}


def build(loops=1, tile_g=21, bufs=4, pool_tiles=0):
    """Paired streaming kernel; tile_g = pair-groups per tile (divides 336).

    The half-size hu2 slab (11MB) is DMA'd once and stays SBUF-resident
    (86KB/partition); only hv streams per pass. pool_tiles>0 would run
    some muls on GPSIMD, but Q7 ucode is far slower on real HW than the
    cost model claims (measured: it regresses) — keep 0. loops > 1 wraps
    the pass in a hardware
    For_i loop (identical output every iteration) so steady-state device
    time can be measured by loop-count differencing inside one NEFF."""
    key = ("p", loops, tile_g, bufs, pool_tiles)
    if key in _BUILT:
        return _BUILT[key]

    f32 = mybir.dt.float32
    bf16 = mybir.dt.bfloat16

    assert NPG % tile_g == 0
    n_tiles = NPG // tile_g

    nc = bacc.Bacc("TRN2", target_bir_lowering=False, debug=False)

    hu_d = nc.dram_tensor("hus", [128, NPG, D_FEAT], bf16, kind="ExternalInput")
    hv_d = nc.dram_tensor("hvs", [128, 2, NPG, D_FEAT], bf16, kind="ExternalInput")
    out_d = nc.dram_tensor("scores", [128, NG], f32, kind="ExternalOutput")

    with tile.TileContext(nc) as tc:
        with (
            tc.tile_pool(name="resid", bufs=1) as rpool,
            tc.tile_pool(name="outp", bufs=1) as outp,
            tc.tile_pool(name="stream", bufs=bufs) as gpool,
            tc.tile_pool(name="scratch", bufs=2) as spool,
        ):
            hu2 = rpool.tile([128, NPG, D_FEAT], bf16)
            nc.sync.dma_start(hu2[:], hu_d[:])
            scores = outp.tile([128, NG], f32)
            scores_v = scores[:].rearrange("p (r g) -> p r g", r=2)

            def body():
                for t in range(n_tiles):
                    g0 = t * tile_g
                    hv = gpool.tile([128, 2, tile_g, D_FEAT], bf16, tag="hv")
                    nc.sync.dma_start(hv[:], hv_d[:, :, g0 : g0 + tile_g, :])
                    prod = spool.tile([128, 2, tile_g, D_FEAT], bf16, tag="prod")
                    hu_sl = hu2[:, g0 : g0 + tile_g, :]
                    eng1 = nc.gpsimd if t >= n_tiles - pool_tiles else nc.vector
                    nc.vector.tensor_mul(prod[:, 0], hu_sl, hv[:, 0])
                    eng1.tensor_mul(prod[:, 1], hu_sl, hv[:, 1])
                    cur = prod
                    w = D_FEAT
                    while w > 2:
                        nxt = spool.tile(
                            [128, 2, tile_g, w // 2], bf16, tag=f"t{w}"
                        )
                        cv = cur[:].rearrange("p r g (h f) -> p r g h f", h=2)
                        nc.vector.tensor_add(
                            nxt[:], cv[:, :, :, 0, :], cv[:, :, :, 1, :]
                        )
                        cur = nxt
                        w //= 2
                    cv = cur[:].rearrange("p r g (h f) -> p r g h f", h=2)
                    nc.vector.tensor_add(
                        scores_v[:, :, g0 : g0 + tile_g],
                        cv[:, :, :, 0, 0],
                        cv[:, :, :, 1, 0],
                    )

            if loops == 1:
                body()
            else:
                with tc.For_i(0, loops, 1):
                    body()
            nc.sync.dma_start(out_d[:], scores[:])

    nc.compile()
    _BUILT[key] = nc
    return nc


def build_flat(loops=1, tile_g=125, bufs=2):
    """Unpaired fallback (no sorting): edge e at [e%128, e//128]."""
    key = ("f", loops, tile_g, bufs)
    if key in _BUILT:
        return _BUILT[key]

    f32 = mybir.dt.float32
    bf16 = mybir.dt.bfloat16

    n_groups = E_PER // 128  # 625
    assert n_groups % tile_g == 0
    n_tiles = n_groups // tile_g

    nc = bacc.Bacc("TRN2", target_bir_lowering=False, debug=False)

    hu_d = nc.dram_tensor("hus", [128, n_groups, D_FEAT], bf16, kind="ExternalInput")
    hv_d = nc.dram_tensor("hvs", [128, n_groups, D_FEAT], bf16, kind="ExternalInput")
    out_d = nc.dram_tensor("scores", [128, n_groups], f32, kind="ExternalOutput")

    with tile.TileContext(nc) as tc:
        with (
            tc.tile_pool(name="outp", bufs=1) as outp,
            tc.tile_pool(name="stream", bufs=bufs) as gpool,
            tc.tile_pool(name="prod", bufs=2) as ppool,
        ):
            scores = outp.tile([128, n_groups], f32)

            def body():
                for t in range(n_tiles):
                    g0 = t * tile_g
                    hu = gpool.tile([128, tile_g, D_FEAT], bf16, tag="hu")
                    hv = gpool.tile([128, tile_g, D_FEAT], bf16, tag="hv")
                    nc.sync.dma_start(hu[:], hu_d[:, g0 : g0 + tile_g, :])
                    nc.sync.dma_start(hv[:], hv_d[:, g0 : g0 + tile_g, :])
                    prod = ppool.tile([128, tile_g, D_FEAT], bf16)
                    nc.vector.tensor_mul(prod[:], hu[:], hv[:])
                    nc.vector.tensor_reduce(
                        scores[:, g0 : g0 + tile_g],
                        prod[:],
                        axis=mybir.AxisListType.X,
                        op=mybir.AluOpType.add,
                    )

            if loops == 1:
                body()
            else:
                with tc.For_i(0, loops, 1):
                    body()
            nc.sync.dma_start(out_d[:], scores[:])

    nc.compile()
    _BUILT[key] = nc
    return nc


def prep_paired(s, d, e2=E2):
    """Sort a core's edges by src, pad equal-src runs to even length.

    Returns (pair_src [e2/2], slot_dst [e2], ed_map [e2]) in
    pair-adjacent order (slots 2i, 2i+1 = pair i), or None on overflow.
    ed_map[j] = original edge index or -1 for padding."""
    n = len(s)
    order = np.argsort(s, kind="stable")
    ss, dd = s[order], d[order]
    change = np.flatnonzero(np.diff(ss)) + 1
    starts = np.concatenate(([0], change))
    ends = np.concatenate((change, [n]))
    lens = ends - starts
    odd = (lens % 2).astype(bool)
    if n + int(odd.sum()) > e2:
        return None
    pads_before = np.concatenate(([0], np.cumsum(odd)[:-1]))
    new_pos = np.arange(n) + np.repeat(pads_before, lens)
    psrc = np.zeros(e2, np.int64)
    pdst = np.zeros(e2, np.int64)
    pedge = np.full(e2, -1, np.int64)
    psrc[new_pos] = ss
    pdst[new_pos] = dd
    pedge[new_pos] = order
    pad_slots = (ends + pads_before)[odd]
    psrc[pad_slots] = ss[ends[odd] - 1]
    return psrc[0::2], pdst, pedge


def make_slabs(h_bf, pair_src, slot_dst):
    """hu2 [128, NPG, 128] (pair p at [p%128, p//128]); hv
    [128, 2, NPG, 128] (r=0: slots 2i, r=1: slots 2i+1)."""
    hu2 = np.ascontiguousarray(
        h_bf[pair_src].reshape(NPG, 128, D_FEAT).transpose(1, 0, 2)
    )
    hv = np.empty((128, 2, NPG, D_FEAT), h_bf.dtype)
    dst_pairs = slot_dst.reshape(NPG * 128, 2)
    for r in range(2):
        hv[:, r] = (
            h_bf[dst_pairs[:, r]].reshape(NPG, 128, D_FEAT).transpose(1, 0, 2)
        )
    return hu2, hv


def unscatter(scores, ed_map):
    """Device scores [128, NG] f32 -> per-original-edge [E_PER] f32."""
    p = np.arange(E2 // 2)
    lane = p % 128
    glo = p // 128
    flat = np.empty(E2, np.float32)
    flat[0::2] = scores[lane, glo]
    flat[1::2] = scores[lane, glo + NPG]
    valid = ed_map >= 0
    out_local = np.empty(E_PER, np.float32)
    out_local[ed_map[valid]] = flat[valid]
    return out_local


def make_slabs_flat(h_bf, src_k, dst_k):
    n_groups = E_PER // 128
    hus = np.ascontiguousarray(
        h_bf[src_k].reshape(n_groups, 128, D_FEAT).transpose(1, 0, 2)
    )
    hvs = np.ascontiguousarray(
        h_bf[dst_k].reshape(n_groups, 128, D_FEAT).transpose(1, 0, 2)
    )
    return hus, hvs


def kernel(h, src, dst):
    h_bf = np.asarray(h, dtype=np.float32).astype(ml_dtypes.bfloat16)
    src = np.asarray(src).astype(np.int64)
    dst = np.asarray(dst).astype(np.int64)

    preps = []
    for k in range(N_CORES):
        sl = slice(k * E_PER, (k + 1) * E_PER)
        preps.append(prep_paired(src[sl], dst[sl]))

    out = np.empty(N_EDGES, np.float32)
    if all(p is not None for p in preps):
        nc = build()
        in_maps = []
        for pair_src, slot_dst, _ in preps:
            hu2, hv = make_slabs(h_bf, pair_src, slot_dst)
            in_maps.append({"hus": hu2, "hvs": hv})
        res = run_bass_kernel_spmd(nc, in_maps, list(range(N_CORES)))
        for k in range(N_CORES):
            sc = res.results[k]["scores"].astype(np.float32)
            out[k * E_PER : (k + 1) * E_PER] = unscatter(sc, preps[k][2])
    else:
        nc = build_flat()
        in_maps = []
        for k in range(N_CORES):
            sl = slice(k * E_PER, (k + 1) * E_PER)
            hus, hvs = make_slabs_flat(h_bf, src[sl], dst[sl])
            in_maps.append({"hus": hus, "hvs": hvs})
        res = run_bass_kernel_spmd(nc, in_maps, list(range(N_CORES)))
        for k in range(N_CORES):
            sc = res.results[k]["scores"].astype(np.float32)
            out[k * E_PER : (k + 1) * E_PER] = sc.T.reshape(-1)
    return out.reshape(N_EDGES, 1)


# revision 6
# speedup vs baseline: 1.4050x; 1.0156x over previous
"""Trainium2 Bass kernel for per-edge dot products (DGL u_dot_v).

score[e] = sum_d h[src[e], d] * h[dst[e], d]   for 640K edges, 10K nodes, D=128.

Strategy (8 NeuronCores, data-parallel over edges, 80K edges/core):

Per-edge gathers on-device are descriptor/ucode-rate bound on this part
(SWDGE dma_gather and gpsimd ap_gather both cost 100s of ns/edge-endpoint
in instruction-issue terms), while the DMA engines stream sequential data
at full rate. So the host lays the gathered operands out as bf16
edge-major slabs in HBM and the device runs a pure streaming pipeline:

  - Host: sort each core's edges by src and pad equal-src runs to even
    length (v1's pairing); pair p shares one hu entry between its two
    edges -> hu slab is half size (25% total DMA saved). Slot layout is
    "halves": pair p = (lane p%128, group p//128) and (same lane,
    group p//128 + NPG), so every device access stays packed-contiguous.
  - DMA: stream hu2 [128, tile, 128] and hv [128, 2, tile, 128] tiles.
  - DVE: two muls (hu2 broadcast across the two halves) in bf16 2x mode.
  - DVE: feature reduction as a binary tree of tensor_tensor adds over
    contiguous half-splits (2x mode per level) — tensor_reduce has no
    fast mode (1 elem/cycle) and would dominate.
  - One f32 scores [128, 672] tile, single DMA out; host inverts the
    permutation.

Measured ~110us/core steady-state on hardware (TimelineSim models 112us);
the DMA stream (33.6MB/core/pass at ~330GB/s) and DVE (~86K cycles) are
both near-saturated.
"""

import sys

import numpy as np

for _p in ("/opt/trn_rl_repo", "/opt/pypackages"):
    if _p not in sys.path:
        sys.path.append(_p)

import ml_dtypes  # noqa: E402

import concourse.mybir as mybir  # noqa: E402
import concourse.tile as tile  # noqa: E402
from concourse import bacc  # noqa: E402
from concourse.bass_utils import run_bass_kernel_spmd  # noqa: E402

N_NODES = 10000
D_FEAT = 128
N_EDGES = 640000
N_CORES = 8
E_PER = N_EDGES // N_CORES  # 80000
E2 = 86016  # padded slots per core (multiple of 256, fits worst pad)
NG = E2 // 128  # 672 slot groups
NPG = NG // 2  # 336 pair groups

_BUILT = {}


def build(loops=1, tile_g=21, bufs=4, pool_tiles=0, stag=False):
    """Paired streaming kernel; tile_g = pair-groups per tile (divides 336).

    The half-size hu2 slab (11MB) is DMA'd once and stays SBUF-resident
    (86KB/partition); only hv streams per pass. pool_tiles>0 would run
    some muls on GPSIMD, but Q7 ucode is far slower on real HW than the
    cost model claims (measured: it regresses) — keep 0. loops > 1 wraps
    the pass in a hardware
    For_i loop (identical output every iteration) so steady-state device
    time can be measured by loop-count differencing inside one NEFF."""
    key = ("p", loops, tile_g, bufs, pool_tiles, stag)
    if key in _BUILT:
        return _BUILT[key]

    f32 = mybir.dt.float32
    bf16 = mybir.dt.bfloat16

    assert NPG % tile_g == 0
    n_tiles = NPG // tile_g

    nc = bacc.Bacc("TRN2", target_bir_lowering=False, debug=False)

    hu_d = nc.dram_tensor("hus", [128, NPG, D_FEAT], bf16, kind="ExternalInput")
    hv_d = nc.dram_tensor("hvs", [128, 2, NPG, D_FEAT], bf16, kind="ExternalInput")
    out_d = nc.dram_tensor("scores", [128, NG], f32, kind="ExternalOutput")

    with tile.TileContext(nc) as tc:
        with (
            tc.tile_pool(name="resid", bufs=1) as rpool,
            tc.tile_pool(name="outp", bufs=1) as outp,
            tc.tile_pool(name="stream", bufs=bufs) as gpool,
            tc.tile_pool(name="scratch", bufs=2) as spool,
        ):
            hu2 = rpool.tile([128, NPG, D_FEAT], bf16)
            nc.sync.dma_start(hu2[:], hu_d[:])
            scores = outp.tile([128, NG], f32)
            scores_v = scores[:].rearrange("p (r g) -> p r g", r=2)

            def body():
                for t in range(n_tiles):
                    g0 = t * tile_g
                    hv = gpool.tile([128, 2, tile_g, D_FEAT], bf16, tag="hv")
                    nc.sync.dma_start(hv[:], hv_d[:, :, g0 : g0 + tile_g, :])
                    prod = spool.tile([128, 2, tile_g, D_FEAT], bf16, tag="prod")
                    hu_sl = hu2[:, g0 : g0 + tile_g, :]
                    eng1 = nc.gpsimd if t >= n_tiles - pool_tiles else nc.vector
                    nc.vector.tensor_mul(prod[:, 0], hu_sl, hv[:, 0])
                    eng1.tensor_mul(prod[:, 1], hu_sl, hv[:, 1])
                    cur = prod
                    w = D_FEAT
                    while w > 2:
                        nxt = spool.tile(
                            [128, 2, tile_g, w // 2], bf16, tag=f"t{w}"
                        )
                        cv = cur[:].rearrange("p r g (h f) -> p r g h f", h=2)
                        nc.vector.tensor_add(
                            nxt[:], cv[:, :, :, 0, :], cv[:, :, :, 1, :]
                        )
                        cur = nxt
                        w //= 2
                    cv = cur[:].rearrange("p r g (h f) -> p r g h f", h=2)
                    nc.vector.tensor_add(
                        scores_v[:, :, g0 : g0 + tile_g],
                        cv[:, :, :, 0, 0],
                        cv[:, :, :, 1, 0],
                    )

            if loops == 1:
                body()
            else:
                with tc.For_i(0, loops, 1, staggered_reset=stag):
                    body()
            nc.sync.dma_start(out_d[:], scores[:])

    nc.compile()
    _BUILT[key] = nc
    return nc


def build_flat(loops=1, tile_g=125, bufs=2):
    """Unpaired fallback (no sorting): edge e at [e%128, e//128]."""
    key = ("f", loops, tile_g, bufs)
    if key in _BUILT:
        return _BUILT[key]

    f32 = mybir.dt.float32
    bf16 = mybir.dt.bfloat16

    n_groups = E_PER // 128  # 625
    assert n_groups % tile_g == 0
    n_tiles = n_groups // tile_g

    nc = bacc.Bacc("TRN2", target_bir_lowering=False, debug=False)

    hu_d = nc.dram_tensor("hus", [128, n_groups, D_FEAT], bf16, kind="ExternalInput")
    hv_d = nc.dram_tensor("hvs", [128, n_groups, D_FEAT], bf16, kind="ExternalInput")
    out_d = nc.dram_tensor("scores", [128, n_groups], f32, kind="ExternalOutput")

    with tile.TileContext(nc) as tc:
        with (
            tc.tile_pool(name="outp", bufs=1) as outp,
            tc.tile_pool(name="stream", bufs=bufs) as gpool,
            tc.tile_pool(name="prod", bufs=2) as ppool,
        ):
            scores = outp.tile([128, n_groups], f32)

            def body():
                for t in range(n_tiles):
                    g0 = t * tile_g
                    hu = gpool.tile([128, tile_g, D_FEAT], bf16, tag="hu")
                    hv = gpool.tile([128, tile_g, D_FEAT], bf16, tag="hv")
                    nc.sync.dma_start(hu[:], hu_d[:, g0 : g0 + tile_g, :])
                    nc.sync.dma_start(hv[:], hv_d[:, g0 : g0 + tile_g, :])
                    prod = ppool.tile([128, tile_g, D_FEAT], bf16)
                    nc.vector.tensor_mul(prod[:], hu[:], hv[:])
                    nc.vector.tensor_reduce(
                        scores[:, g0 : g0 + tile_g],
                        prod[:],
                        axis=mybir.AxisListType.X,
                        op=mybir.AluOpType.add,
                    )

            if loops == 1:
                body()
            else:
                with tc.For_i(0, loops, 1):
                    body()
            nc.sync.dma_start(out_d[:], scores[:])

    nc.compile()
    _BUILT[key] = nc
    return nc


def prep_paired(s, d, e2=E2):
    """Sort a core's edges by src, pad equal-src runs to even length.

    Returns (pair_src [e2/2], slot_dst [e2], ed_map [e2]) in
    pair-adjacent order (slots 2i, 2i+1 = pair i), or None on overflow.
    ed_map[j] = original edge index or -1 for padding."""
    n = len(s)
    order = np.argsort(s, kind="stable")
    ss, dd = s[order], d[order]
    change = np.flatnonzero(np.diff(ss)) + 1
    starts = np.concatenate(([0], change))
    ends = np.concatenate((change, [n]))
    lens = ends - starts
    odd = (lens % 2).astype(bool)
    if n + int(odd.sum()) > e2:
        return None
    pads_before = np.concatenate(([0], np.cumsum(odd)[:-1]))
    new_pos = np.arange(n) + np.repeat(pads_before, lens)
    psrc = np.zeros(e2, np.int64)
    pdst = np.zeros(e2, np.int64)
    pedge = np.full(e2, -1, np.int64)
    psrc[new_pos] = ss
    pdst[new_pos] = dd
    pedge[new_pos] = order
    pad_slots = (ends + pads_before)[odd]
    psrc[pad_slots] = ss[ends[odd] - 1]
    return psrc[0::2], pdst, pedge


def make_slabs(h_bf, pair_src, slot_dst):
    """hu2 [128, NPG, 128] (pair p at [p%128, p//128]); hv
    [128, 2, NPG, 128] (r=0: slots 2i, r=1: slots 2i+1)."""
    hu2 = np.ascontiguousarray(
        h_bf[pair_src].reshape(NPG, 128, D_FEAT).transpose(1, 0, 2)
    )
    hv = np.empty((128, 2, NPG, D_FEAT), h_bf.dtype)
    dst_pairs = slot_dst.reshape(NPG * 128, 2)
    for r in range(2):
        hv[:, r] = (
            h_bf[dst_pairs[:, r]].reshape(NPG, 128, D_FEAT).transpose(1, 0, 2)
        )
    return hu2, hv


def unscatter(scores, ed_map):
    """Device scores [128, NG] f32 -> per-original-edge [E_PER] f32."""
    p = np.arange(E2 // 2)
    lane = p % 128
    glo = p // 128
    flat = np.empty(E2, np.float32)
    flat[0::2] = scores[lane, glo]
    flat[1::2] = scores[lane, glo + NPG]
    valid = ed_map >= 0
    out_local = np.empty(E_PER, np.float32)
    out_local[ed_map[valid]] = flat[valid]
    return out_local


def make_slabs_flat(h_bf, src_k, dst_k):
    n_groups = E_PER // 128
    hus = np.ascontiguousarray(
        h_bf[src_k].reshape(n_groups, 128, D_FEAT).transpose(1, 0, 2)
    )
    hvs = np.ascontiguousarray(
        h_bf[dst_k].reshape(n_groups, 128, D_FEAT).transpose(1, 0, 2)
    )
    return hus, hvs


def kernel(h, src, dst):
    h_bf = np.asarray(h, dtype=np.float32).astype(ml_dtypes.bfloat16)
    src = np.asarray(src).astype(np.int64)
    dst = np.asarray(dst).astype(np.int64)

    preps = []
    for k in range(N_CORES):
        sl = slice(k * E_PER, (k + 1) * E_PER)
        preps.append(prep_paired(src[sl], dst[sl]))

    out = np.empty(N_EDGES, np.float32)
    if all(p is not None for p in preps):
        nc = build()
        in_maps = []
        for pair_src, slot_dst, _ in preps:
            hu2, hv = make_slabs(h_bf, pair_src, slot_dst)
            in_maps.append({"hus": hu2, "hvs": hv})
        res = run_bass_kernel_spmd(nc, in_maps, list(range(N_CORES)))
        for k in range(N_CORES):
            sc = res.results[k]["scores"].astype(np.float32)
            out[k * E_PER : (k + 1) * E_PER] = unscatter(sc, preps[k][2])
    else:
        nc = build_flat()
        in_maps = []
        for k in range(N_CORES):
            sl = slice(k * E_PER, (k + 1) * E_PER)
            hus, hvs = make_slabs_flat(h_bf, src[sl], dst[sl])
            in_maps.append({"hus": hus, "hvs": hvs})
        res = run_bass_kernel_spmd(nc, in_maps, list(range(N_CORES)))
        for k in range(N_CORES):
            sc = res.results[k]["scores"].astype(np.float32)
            out[k * E_PER : (k + 1) * E_PER] = sc.T.reshape(-1)
    return out.reshape(N_EDGES, 1)
